# revision 16
# baseline (speedup 1.0000x reference)
"""Trainium2 Bass kernel for nn_NeuralNetwork_31447750541324.

Network: per-frame conv stack (stride==kernel convs -> pure matmuls) ->
BatchNorm1d over (B, len) -> per-sample channel reorder by range ->
3 Elman RNNs (input 1, hidden 256) over F=64 steps -> mean -> linear.

Sharding: launch A runs the conv stack data-parallel over the 640 frames
(80 frames/core on 8 cores).  The tiny [640,3] conv result is re-arranged
on host (BN stats + affine, range argsort, channel select: ~10k FLOPs),
then launch B runs the 3 RNNs on 3 cores (one RNN each) including the
final linear projection; host sums the 3 partial projections + bias.
"""

import os
import numpy as np

# ---------------- static problem dims ----------------
B, F, C, H, W = 10, 64, 3, 180, 180
NF = B * F                      # 640 frames
NCORES = 8
FPC = NF // NCORES              # 80 frames per core
CH, OUT, NCLS = 64, 256, 5
K1, K2 = 9, 9                   # conv1 kernel (9x9, stride 9)
KC1 = C * 9 * 9                 # 243 contraction
KC1P = 256                      # padded to 2 chunks of 128
N1 = 400                        # 20x20 conv1 output positions
EPS = 1e-5

_cache = {}


def _f16(a):
    return np.ascontiguousarray(a, dtype=np.float16)


def _f32(a):
    return np.ascontiguousarray(a, dtype=np.float32)


# ---------------- launch A: conv stack, 8 cores ----------------
# 8-frame DMA groups; 2 frames packed per PSUM tile via column-tiled
# matmuls (partitions 0-63 = even frame, 64-127 = odd frame); ACT does
# relu+bias from PSUM, DVE maxpools in fp16; conv2 split in two halves.
GRP = 4           # frames per DMA group
NGRP = FPC // GRP
NPAIR = FPC // 2  # 40 psum pairs
NQ = 4            # conv2 split into quarters


def _build_conv_nc():
    import concourse.bacc as bacc
    import concourse.bass as bass
    import concourse.mybir as mybir
    import concourse.tile as tile

    f16, f32 = mybir.dt.float16, mybir.dt.float32
    nc = bacc.Bacc("TRN2", target_bir_lowering=False, debug=False,
                   num_devices=NCORES)

    p1 = nc.dram_tensor("p1", [NGRP, 128, GRP, 2, N1], f16,
                        kind="ExternalInput")
    w1 = nc.dram_tensor("w1", [128, 2, 128], f16, kind="ExternalInput")
    w2 = nc.dram_tensor("w2", [128, 25, 3], f16, kind="ExternalInput")
    b1 = nc.dram_tensor("b1", [128, 1], f32, kind="ExternalInput")
    b2 = nc.dram_tensor("b2", [3, 1], f32, kind="ExternalInput")
    yp = nc.dram_tensor("ypart", [3, NQ, 2, NPAIR // NQ], f32,
                        kind="ExternalOutput")

    Relu = mybir.ActivationFunctionType.Relu
    X, XY = mybir.AxisListType.X, mybir.AxisListType.XY
    mx = mybir.AluOpType.max

    with tile.TileContext(nc) as tc:
        with (
            tc.tile_pool(name="const", bufs=1) as cp,
            tc.tile_pool(name="frames", bufs=3) as fp,
            tc.tile_pool(name="red", bufs=4) as rp,
            tc.tile_pool(name="ps1", bufs=4, space=bass.MemorySpace.PSUM) as pp1,
            tc.tile_pool(name="ps2", bufs=2, space=bass.MemorySpace.PSUM) as pp2,
        ):
            w1s = cp.tile([128, 2, 128], f16, tag="w1")
            w2s = cp.tile([128, 25, 3], f16, tag="w2")
            b1s = cp.tile([128, 1], f32, tag="b1")
            b2s = cp.tile([3, 1], f32, tag="b2")
            pool1 = cp.tile([128, NPAIR, 100], f16, tag="pool1")
            yo = cp.tile([3, NQ, 2, NPAIR // NQ], f32, tag="yo")
            # consts go on gpsimd's queue so the sync engine's very first
            # instruction is the group-0 frame DMA
            nc.gpsimd.dma_start(w1s[:], w1[:])
            nc.gpsimd.dma_start(w2s[:], w2[:])
            nc.gpsimd.dma_start(b1s[:], b1[:])
            nc.gpsimd.dma_start(b2s[:], b2[:])

            pv = pool1[:].rearrange("p q (a x b y) -> p q a x b y",
                                    a=2, x=5, b=2)

            def conv2_quarter(h):
                # pairs [10h, 10h+10); even frames from partitions 0:64,
                # odd frames from partitions 64:128 (row-tiled K=64).
                npq = NPAIR // NQ
                sl = slice(npq * h, npq * (h + 1))
                pse = pp2.tile([3, npq, 2, 2], f32, tag="ps2e")
                pso = pp2.tile([3, npq, 2, 2], f32, tag="ps2o")
                for j in range(25):
                    kh, kw = j // 5, j % 5
                    nc.tensor.matmul(pse[:], w2s[0:64, j, :],
                                     pv[0:64, sl, :, kh, :, kw],
                                     start=(j == 0), stop=(j == 24))
                    nc.tensor.matmul(pso[:], w2s[64:128, j, :],
                                     pv[64:128, sl, :, kh, :, kw],
                                     start=(j == 0), stop=(j == 24))
                for par, psx in ((0, pse), (1, pso)):
                    rt2 = rp.tile([3, npq], f32, tag="rt2")
                    nc.vector.tensor_reduce(rt2[:], psx[:], axis=XY, op=mx)
                    nc.scalar.activation(yo[:, h, par, :], rt2[:],
                                         Relu, bias=b2s[:])

            for g in range(NGRP):
                gt = fp.tile([128, GRP, 2, N1], f16, tag="fr")
                nc.sync.dma_start(gt[:], p1[g])
                for p in range(GRP // 2):
                    fa, fb = 2 * p, 2 * p + 1
                    ps = pp1.tile([128, 100, 4], f32, tag="ps")
                    nc.tensor.matmul(ps[0:64], w1s[:, 0, 0:64],
                                     gt[:, fa, 0, :], start=True, stop=False)
                    nc.tensor.matmul(ps[64:128], w1s[:, 0, 64:128],
                                     gt[:, fb, 0, :], start=True, stop=False)
                    nc.tensor.matmul(ps[0:64], w1s[:, 1, 0:64],
                                     gt[:, fa, 1, :], start=False, stop=True)
                    nc.tensor.matmul(ps[64:128], w1s[:, 1, 64:128],
                                     gt[:, fb, 1, :], start=False, stop=True)
                    rt = rp.tile([128, 100], f32, tag="rt")
                    nc.vector.tensor_reduce(rt[:], ps[:], axis=X, op=mx)
                    nc.scalar.activation(pool1[:, g * (GRP // 2) + p, :],
                                         rt[:], Relu, bias=b1s[:])
                if g in (4, 9, 14):
                    conv2_quarter(g // 5)
            conv2_quarter(NQ - 1)
            nc.sync.dma_start(yp[:], yo[:])

    nc.compile()
    return nc


# ---------------- launch B: one RNN per core, 3 cores ----------------
def _build_rnn_nc():
    import concourse.bacc as bacc
    import concourse.bass as bass
    import concourse.mybir as mybir
    import concourse.tile as tile

    f16, f32 = mybir.dt.float16, mybir.dt.float32
    nc = bacc.Bacc("TRN2", target_bir_lowering=False, debug=False,
                   num_devices=3)

    xb = nc.dram_tensor("xb", [128, F, B], f16, kind="ExternalInput")
    wh = nc.dram_tensor("whht", [128, 2, 2, 128], f16, kind="ExternalInput")
    cf = nc.dram_tensor("cfw", [128, 2, 128], f16, kind="ExternalInput")
    wl = nc.dram_tensor("wl3", [128, 2, 5], f16, kind="ExternalInput")
    pr = nc.dram_tensor("pr", [B, NCLS], f32, kind="ExternalOutput")

    Tanh = mybir.ActivationFunctionType.Tanh

    with tile.TileContext(nc) as tc:
        with (
            tc.tile_pool(name="const", bufs=1) as cp,
            tc.tile_pool(name="h", bufs=2) as hp,
            tc.tile_pool(name="ps", bufs=4, space=bass.MemorySpace.PSUM) as pp,
        ):
            xbs = cp.tile([128, F, B], f16, tag="xb")
            whs = cp.tile([128, 2, 2, 128], f16, tag="wh")
            cfs = cp.tile([128, 2, 128], f16, tag="cf")
            wls = cp.tile([128, 2, 5], f16, tag="wl")
            nc.sync.dma_start(xbs[:], xb[:])
            nc.sync.dma_start(whs[:], wh[:])
            nc.sync.dma_start(cfs[:], cf[:])
            nc.sync.dma_start(wls[:], wl[:])

            h = None
            for t in range(F):
                ps = pp.tile([128, 2, B], f32, tag="ps")
                for mc in range(2):
                    nc.tensor.matmul(ps[:, mc, :], cfs[:, mc, :],
                                     xbs[:, t, :], start=True, stop=(t == 0))
                    if t > 0:
                        nc.tensor.matmul(ps[:, mc, :], whs[:, 0, mc, :],
                                         h[:, 0, :], start=False, stop=False)
                        nc.tensor.matmul(ps[:, mc, :], whs[:, 1, mc, :],
                                         h[:, 1, :], start=False, stop=True)
                ht = hp.tile([128, 2, B], f16, tag="h")
                nc.scalar.activation(ht[:], ps[:], Tanh)
                h = ht

            psf = pp.tile([B, NCLS], f32, tag="psf")
            nc.tensor.matmul(psf[:], h[:, 0, :], wls[:, 0, :],
                             start=True, stop=False)
            nc.tensor.matmul(psf[:], h[:, 1, :], wls[:, 1, :],
                             start=False, stop=True)
            po = cp.tile([B, NCLS], f32, tag="po")
            nc.vector.tensor_copy(po[:], psf[:])
            nc.sync.dma_start(pr[:], po[:])

    nc.compile()
    return nc


# ---------------- host-side input prep ----------------
def _prep_conv_inputs(x, W1, b1, W2, b2):
    # im2col for conv1: stride==kernel => non-overlapping patches.
    # n-order (oh10, ow10, ph, pw) groups each 2x2 maxpool window in the
    # last free axis; k-order (c, kh, kw) matches W1 flattening.
    xv = x.reshape(NF, C, 10, 2, 9, 20, 9)          # (fr,c,oh10,ph,kh,w,kw)
    xv = xv.reshape(NF, C, 10, 2, 9, 10, 2, 9)      # split w -> (ow10,pw)
    pat = xv.transpose(0, 1, 4, 7, 2, 5, 3, 6).reshape(NF, KC1, N1)
    patp = np.zeros((NF, KC1P, N1), np.float16)
    patp[:, :KC1] = pat
    # [NF, 128, 2, N1], then group GRP frames per DMA: [NC, NGRP, 128, GRP, 2, N1]
    p1 = patp.reshape(NF, 2, 128, N1).transpose(0, 2, 1, 3)
    p1 = p1.reshape(NCORES, NGRP, GRP, 128, 2, N1).transpose(0, 1, 3, 2, 4, 5)
    p1 = np.ascontiguousarray(p1)

    w1m = np.zeros((KC1P, 64), np.float16)
    w1m[:KC1] = W1.reshape(64, KC1).T               # [K, M]
    w1c = w1m.reshape(2, 128, 64).transpose(1, 0, 2)  # [128, 2, 64]
    w1t = np.concatenate([w1c, w1c], axis=2)        # [128, 2, 128] dup cols
    w1t = np.ascontiguousarray(w1t)

    # conv2 lhsT per (kh,kw): [64, 3], duplicated on rows for odd frames
    w2c = W2.transpose(1, 2, 3, 0).reshape(64, 25, 3).astype(np.float16)
    w2t = np.ascontiguousarray(np.concatenate([w2c, w2c], axis=0))

    b1d = np.concatenate([b1, b1]).reshape(128, 1)
    return p1, w1t, w2t, _f32(b1d), _f32(b2.reshape(3, 1))


def _prep_rnn_inputs(ts_r, Wih_r, Whh_r, bih_r, bhh_r, Wl):
    # ts_r: [F, B] f32 rank-r input sequence
    xbv = np.zeros((128, F, B), np.float16)
    xbv[0] = ts_r
    xbv[1] = 1.0
    wht = np.zeros((128, 2, 2, 128), np.float16)
    WhhT = Whh_r.T                                   # [k, m]
    for kc in range(2):
        for mc in range(2):
            wht[:, kc, mc, :] = WhhT[kc * 128:(kc + 1) * 128,
                                     mc * 128:(mc + 1) * 128]
    cfw = np.zeros((128, 2, 128), np.float16)
    bsum = bih_r + bhh_r
    for mc in range(2):
        cfw[0, mc, :] = Wih_r[mc * 128:(mc + 1) * 128, 0]
        cfw[1, mc, :] = bsum[mc * 128:(mc + 1) * 128]
    wl3 = np.zeros((128, 2, 5), np.float16)
    WlT3 = (Wl.T / 3.0)                              # [256, 5]
    for kc in range(2):
        wl3[:, kc, :] = WlT3[kc * 128:(kc + 1) * 128]
    return xbv, wht, cfw, wl3


def _ensure_profile_hook():
    """antenv.axon_hooks is absent in this image; synthesize it so
    run_bass_kernel_spmd(trace=True) can capture NTFF profiles."""
    import sys
    import types
    try:
        from antenv.axon_hooks import get_axon_ntff_profile_hook  # noqa
        return True
    except ImportError:
        pass
    try:
        sys.path.insert(0, "/root/.axon_site/trn_agent_boot")
        from trn_boot import _ntff_profile_via_ctypes
        hook = _ntff_profile_via_ctypes("/opt/axon/libaxon_pjrt.so")
        if hook is None:
            return False
        import antenv
        mod = types.ModuleType("antenv.axon_hooks")
        mod._hook = hook
        mod.get_axon_ntff_profile_hook = lambda: mod._hook
        mod.set_axon_ntff_profile_hook = lambda h: setattr(mod, "_hook", h)
        sys.modules["antenv.axon_hooks"] = mod
        antenv.axon_hooks = mod
        return True
    except Exception:
        return False


def _run(nc, in_maps, core_ids, label):
    from concourse.bass_utils import run_bass_kernel_spmd
    trace = os.environ.get("KERNEL_TRACE", "0") == "1"
    if trace:
        trace = _ensure_profile_hook()
    kw = {}
    if trace:
        import tempfile
        tdir = tempfile.mkdtemp(prefix=f"ktrace_{label}_")
        kw = {"tmpdir": tdir}
    res = run_bass_kernel_spmd(nc, in_maps, core_ids, trace=trace, **kw)
    _cache.setdefault("exec_ns", {})[label] = res.exec_time_ns
    _cache.setdefault("results_obj", {})[label] = res
    return res.results


# ---------------- main entry ----------------
def kernel(x, W1, b1, W2, b2, gamma, beta, Wih, Whh, bih, bhh, Wl, bl):
    x, W1, b1, W2, b2 = map(np.asarray, (x, W1, b1, W2, b2))
    gamma, beta = np.asarray(gamma), np.asarray(beta)
    Wih, Whh, bih, bhh = map(np.asarray, (Wih, Whh, bih, bhh))
    Wl, bl = np.asarray(Wl), np.asarray(bl)

    if "conv" not in _cache:
        _cache["conv"] = _build_conv_nc()
    if "rnn" not in _cache:
        _cache["rnn"] = _build_rnn_nc()

    # ---- launch A: conv stack over 640 frames on 8 cores ----
    p1, w1t, w2t, b1c, b2c = _prep_conv_inputs(x, W1, b1, W2, b2)
    in_maps = [
        {"p1": p1[k], "w1": w1t, "w2": w2t, "b1": b1c, "b2": b2c}
        for k in range(NCORES)
    ]
    res = _run(_cache["conv"], in_maps, list(range(NCORES)), "conv")
    # ypart [3, NQ, 2parity, npq]: frame f = 2*(npq*h + i) + par
    npq = NPAIR // NQ
    y = np.empty((NF, 3), np.float32)
    for k, r in enumerate(res):
        yp = r["ypart"]
        fr = np.empty((FPC, 3), np.float32)
        for hh in range(NQ):
            for par in range(2):
                idx = 2 * (npq * hh + np.arange(npq)) + par
                fr[idx] = yp[:, hh, par, :].T
        y[k * FPC:(k + 1) * FPC] = fr
    y = y.reshape(B, F, 3)

    # ---- host glue: BN (train-mode) + per-sample channel reorder ----
    mean = y.mean(axis=(0, 2), keepdims=True)
    var = y.var(axis=(0, 2), keepdims=True)
    yn = (y - mean) / np.sqrt(var + EPS) * gamma[None, :, None] \
        + beta[None, :, None]
    t = yn.transpose(0, 2, 1)                        # [B, 3, F]
    rng = t.max(-1) - t.min(-1)
    perm = np.argsort(rng, axis=1, kind="stable")
    tsel = np.take_along_axis(t, perm[:, :, None], axis=1)  # [B, 3, F]

    # ---- launch B: 3 RNNs on 3 cores (+ scaled final linear) ----
    in_maps_b = []
    for r in range(3):
        ts_r = tsel[:, r, :].T                       # [F, B]
        xbv, wht, cfw, wl3 = _prep_rnn_inputs(
            ts_r, Wih[r], Whh[r], bih[r], bhh[r], Wl)
        in_maps_b.append({"xb": xbv, "whht": wht, "cfw": cfw, "wl3": wl3})
    res_b = _run(_cache["rnn"], in_maps_b, [0, 1, 2], "rnn")

    out = res_b[0]["pr"] + res_b[1]["pr"] + res_b[2]["pr"] + bl[None, :]
    return out.astype(np.float32)


# revision 18
# speedup vs baseline: 1.0702x; 1.0702x over previous
"""Trainium2 Bass kernel for nn_NeuralNetwork_31447750541324.

Network: per-frame conv stack (stride==kernel convs -> pure matmuls) ->
BatchNorm1d over (B, len) -> per-sample channel reorder by range ->
3 Elman RNNs (input 1, hidden 256) over F=64 steps -> mean -> linear.

Sharding: launch A runs the conv stack data-parallel over the 640 frames
(80 frames/core on 8 cores).  The tiny [640,3] conv result is re-arranged
on host (BN stats + affine, range argsort, channel select: ~10k FLOPs),
then launch B runs the 3 RNNs on 3 cores (one RNN each) including the
final linear projection; host sums the 3 partial projections + bias.
"""

import os
import numpy as np

# ---------------- static problem dims ----------------
B, F, C, H, W = 10, 64, 3, 180, 180
NF = B * F                      # 640 frames
NCORES = 8
FPC = NF // NCORES              # 80 frames per core
CH, OUT, NCLS = 64, 256, 5
K1, K2 = 9, 9                   # conv1 kernel (9x9, stride 9)
KC1 = C * 9 * 9                 # 243 contraction
KC1P = 256                      # padded to 2 chunks of 128
N1 = 400                        # 20x20 conv1 output positions
EPS = 1e-5

_cache = {}


def _f16(a):
    return np.ascontiguousarray(a, dtype=np.float16)


def _f32(a):
    return np.ascontiguousarray(a, dtype=np.float32)


# ---------------- launch A: conv stack, 8 cores ----------------
# 8-frame DMA groups; 2 frames packed per PSUM tile via column-tiled
# matmuls (partitions 0-63 = even frame, 64-127 = odd frame); ACT does
# relu+bias from PSUM, DVE maxpools in fp16; conv2 split in two halves.
GRP = 4           # frames per DMA group
NGRP = FPC // GRP
NPAIR = FPC // 2  # 40 psum pairs
NQ = 4            # conv2 split into quarters


def _build_conv_nc():
    import concourse.bacc as bacc
    import concourse.bass as bass
    import concourse.mybir as mybir
    import concourse.tile as tile

    f16, f32 = mybir.dt.float16, mybir.dt.float32
    nc = bacc.Bacc("TRN2", target_bir_lowering=False, debug=False,
                   num_devices=NCORES)

    p1 = nc.dram_tensor("p1", [NGRP, 128, GRP, 2, N1], f16,
                        kind="ExternalInput")
    w1 = nc.dram_tensor("w1", [128, 2, 128], f16, kind="ExternalInput")
    w2 = nc.dram_tensor("w2", [128, 25, 3], f16, kind="ExternalInput")
    b1 = nc.dram_tensor("b1", [128, 1], f32, kind="ExternalInput")
    b2 = nc.dram_tensor("b2", [3, 1], f32, kind="ExternalInput")
    yp = nc.dram_tensor("ypart", [3, NQ, 2, NPAIR // NQ], f32,
                        kind="ExternalOutput")

    Relu = mybir.ActivationFunctionType.Relu
    X, XY = mybir.AxisListType.X, mybir.AxisListType.XY
    mx = mybir.AluOpType.max

    with tile.TileContext(nc) as tc:
        with (
            tc.tile_pool(name="const", bufs=1) as cp,
            tc.tile_pool(name="frames", bufs=6) as fp,
            tc.tile_pool(name="red", bufs=4) as rp,
            tc.tile_pool(name="ps1", bufs=4, space=bass.MemorySpace.PSUM) as pp1,
            tc.tile_pool(name="ps2", bufs=2, space=bass.MemorySpace.PSUM) as pp2,
        ):
            w1s = cp.tile([128, 2, 128], f16, tag="w1")
            w2s = cp.tile([128, 25, 3], f16, tag="w2")
            b1s = cp.tile([128, 1], f32, tag="b1")
            b2s = cp.tile([3, 1], f32, tag="b2")
            pool1 = cp.tile([128, NPAIR, 100], f16, tag="pool1")
            yo = cp.tile([3, NQ, 2, NPAIR // NQ], f32, tag="yo")
            # consts go on gpsimd's queue so the sync engine's very first
            # instruction is the group-0 frame DMA
            nc.scalar.dma_start(w1s[:], w1[:])
            nc.scalar.dma_start(w2s[:], w2[:])
            nc.scalar.dma_start(b1s[:], b1[:])
            nc.scalar.dma_start(b2s[:], b2[:])

            pv = pool1[:].rearrange("p q (a x b y) -> p q a x b y",
                                    a=2, x=5, b=2)

            def conv2_quarter(h):
                # pairs [10h, 10h+10); even frames from partitions 0:64,
                # odd frames from partitions 64:128 (row-tiled K=64).
                npq = NPAIR // NQ
                sl = slice(npq * h, npq * (h + 1))
                pse = pp2.tile([3, npq, 2, 2], f32, tag="ps2e")
                pso = pp2.tile([3, npq, 2, 2], f32, tag="ps2o")
                for j in range(25):
                    kh, kw = j // 5, j % 5
                    nc.tensor.matmul(pse[:], w2s[0:64, j, :],
                                     pv[0:64, sl, :, kh, :, kw],
                                     start=(j == 0), stop=(j == 24))
                    nc.tensor.matmul(pso[:], w2s[64:128, j, :],
                                     pv[64:128, sl, :, kh, :, kw],
                                     start=(j == 0), stop=(j == 24))
                for par, psx in ((0, pse), (1, pso)):
                    rt2 = rp.tile([3, npq], f32, tag="rt2")
                    nc.vector.tensor_reduce(rt2[:], psx[:], axis=XY, op=mx)
                    nc.scalar.activation(yo[:, h, par, :], rt2[:],
                                         Relu, bias=b2s[:])

            for g in range(NGRP):
                gt = fp.tile([128, GRP, 2, N1], f16, tag="fr")
                nc.sync.dma_start(gt[:], p1[g])
                for p in range(GRP // 2):
                    fa, fb = 2 * p, 2 * p + 1
                    ps = pp1.tile([128, 100, 4], f32, tag="ps")
                    nc.tensor.matmul(ps[0:64], w1s[:, 0, 0:64],
                                     gt[:, fa, 0, :], start=True, stop=False)
                    nc.tensor.matmul(ps[64:128], w1s[:, 0, 64:128],
                                     gt[:, fb, 0, :], start=True, stop=False)
                    nc.tensor.matmul(ps[0:64], w1s[:, 1, 0:64],
                                     gt[:, fa, 1, :], start=False, stop=True)
                    nc.tensor.matmul(ps[64:128], w1s[:, 1, 64:128],
                                     gt[:, fb, 1, :], start=False, stop=True)
                    rt = rp.tile([128, 100], f32, tag="rt")
                    nc.vector.tensor_reduce(rt[:], ps[:], axis=X, op=mx)
                    nc.scalar.activation(pool1[:, g * (GRP // 2) + p, :],
                                         rt[:], Relu, bias=b1s[:])
                if g in (4, 9, 14):
                    conv2_quarter(g // 5)
            conv2_quarter(NQ - 1)
            nc.sync.dma_start(yp[:], yo[:])

    nc.compile()
    return nc


# ---------------- launch B: one RNN per core, 3 cores ----------------
def _build_rnn_nc():
    import concourse.bacc as bacc
    import concourse.bass as bass
    import concourse.mybir as mybir
    import concourse.tile as tile

    f16, f32 = mybir.dt.float16, mybir.dt.float32
    nc = bacc.Bacc("TRN2", target_bir_lowering=False, debug=False,
                   num_devices=3)

    xb = nc.dram_tensor("xb", [128, F, B], f16, kind="ExternalInput")
    wh = nc.dram_tensor("whht", [128, 2, 2, 128], f16, kind="ExternalInput")
    cf = nc.dram_tensor("cfw", [128, 2, 128], f16, kind="ExternalInput")
    wl = nc.dram_tensor("wl3", [128, 2, 5], f16, kind="ExternalInput")
    pr = nc.dram_tensor("pr", [B, NCLS], f32, kind="ExternalOutput")

    Tanh = mybir.ActivationFunctionType.Tanh

    with tile.TileContext(nc) as tc:
        with (
            tc.tile_pool(name="const", bufs=1) as cp,
            tc.tile_pool(name="h", bufs=2) as hp,
            tc.tile_pool(name="ps", bufs=4, space=bass.MemorySpace.PSUM) as pp,
        ):
            xbs = cp.tile([128, F, B], f16, tag="xb")
            whs = cp.tile([128, 2, 2, 128], f16, tag="wh")
            cfs = cp.tile([128, 2, 128], f16, tag="cf")
            wls = cp.tile([128, 2, 5], f16, tag="wl")
            nc.sync.dma_start(xbs[:], xb[:])
            nc.sync.dma_start(whs[:], wh[:])
            nc.sync.dma_start(cfs[:], cf[:])
            nc.sync.dma_start(wls[:], wl[:])

            h = None
            for t in range(F):
                ps = pp.tile([128, 2, B], f32, tag="ps")
                for mc in range(2):
                    nc.tensor.matmul(ps[:, mc, :], cfs[:, mc, :],
                                     xbs[:, t, :], start=True, stop=(t == 0))
                    if t > 0:
                        nc.tensor.matmul(ps[:, mc, :], whs[:, 0, mc, :],
                                         h[:, 0, :], start=False, stop=False)
                        nc.tensor.matmul(ps[:, mc, :], whs[:, 1, mc, :],
                                         h[:, 1, :], start=False, stop=True)
                ht = hp.tile([128, 2, B], f16, tag="h")
                nc.scalar.activation(ht[:], ps[:], Tanh)
                h = ht

            psf = pp.tile([B, NCLS], f32, tag="psf")
            nc.tensor.matmul(psf[:], h[:, 0, :], wls[:, 0, :],
                             start=True, stop=False)
            nc.tensor.matmul(psf[:], h[:, 1, :], wls[:, 1, :],
                             start=False, stop=True)
            po = cp.tile([B, NCLS], f32, tag="po")
            nc.vector.tensor_copy(po[:], psf[:])
            nc.sync.dma_start(pr[:], po[:])

    nc.compile()
    return nc


# ---------------- host-side input prep ----------------
def _prep_conv_inputs(x, W1, b1, W2, b2):
    # im2col for conv1: stride==kernel => non-overlapping patches.
    # n-order (oh10, ow10, ph, pw) groups each 2x2 maxpool window in the
    # last free axis; k-order (c, kh, kw) matches W1 flattening.
    xv = x.reshape(NF, C, 10, 2, 9, 20, 9)          # (fr,c,oh10,ph,kh,w,kw)
    xv = xv.reshape(NF, C, 10, 2, 9, 10, 2, 9)      # split w -> (ow10,pw)
    pat = xv.transpose(0, 1, 4, 7, 2, 5, 3, 6).reshape(NF, KC1, N1)
    patp = np.zeros((NF, KC1P, N1), np.float16)
    patp[:, :KC1] = pat
    # [NF, 128, 2, N1], then group GRP frames per DMA: [NC, NGRP, 128, GRP, 2, N1]
    p1 = patp.reshape(NF, 2, 128, N1).transpose(0, 2, 1, 3)
    p1 = p1.reshape(NCORES, NGRP, GRP, 128, 2, N1).transpose(0, 1, 3, 2, 4, 5)
    p1 = np.ascontiguousarray(p1)

    w1m = np.zeros((KC1P, 64), np.float16)
    w1m[:KC1] = W1.reshape(64, KC1).T               # [K, M]
    w1c = w1m.reshape(2, 128, 64).transpose(1, 0, 2)  # [128, 2, 64]
    w1t = np.concatenate([w1c, w1c], axis=2)        # [128, 2, 128] dup cols
    w1t = np.ascontiguousarray(w1t)

    # conv2 lhsT per (kh,kw): [64, 3], duplicated on rows for odd frames
    w2c = W2.transpose(1, 2, 3, 0).reshape(64, 25, 3).astype(np.float16)
    w2t = np.ascontiguousarray(np.concatenate([w2c, w2c], axis=0))

    b1d = np.concatenate([b1, b1]).reshape(128, 1)
    return p1, w1t, w2t, _f32(b1d), _f32(b2.reshape(3, 1))


def _prep_rnn_inputs(ts_r, Wih_r, Whh_r, bih_r, bhh_r, Wl):
    # ts_r: [F, B] f32 rank-r input sequence
    xbv = np.zeros((128, F, B), np.float16)
    xbv[0] = ts_r
    xbv[1] = 1.0
    wht = np.zeros((128, 2, 2, 128), np.float16)
    WhhT = Whh_r.T                                   # [k, m]
    for kc in range(2):
        for mc in range(2):
            wht[:, kc, mc, :] = WhhT[kc * 128:(kc + 1) * 128,
                                     mc * 128:(mc + 1) * 128]
    cfw = np.zeros((128, 2, 128), np.float16)
    bsum = bih_r + bhh_r
    for mc in range(2):
        cfw[0, mc, :] = Wih_r[mc * 128:(mc + 1) * 128, 0]
        cfw[1, mc, :] = bsum[mc * 128:(mc + 1) * 128]
    wl3 = np.zeros((128, 2, 5), np.float16)
    WlT3 = (Wl.T / 3.0)                              # [256, 5]
    for kc in range(2):
        wl3[:, kc, :] = WlT3[kc * 128:(kc + 1) * 128]
    return xbv, wht, cfw, wl3


def _ensure_profile_hook():
    """antenv.axon_hooks is absent in this image; synthesize it so
    run_bass_kernel_spmd(trace=True) can capture NTFF profiles."""
    import sys
    import types
    try:
        from antenv.axon_hooks import get_axon_ntff_profile_hook  # noqa
        return True
    except ImportError:
        pass
    try:
        sys.path.insert(0, "/root/.axon_site/trn_agent_boot")
        from trn_boot import _ntff_profile_via_ctypes
        hook = _ntff_profile_via_ctypes("/opt/axon/libaxon_pjrt.so")
        if hook is None:
            return False
        import antenv
        mod = types.ModuleType("antenv.axon_hooks")
        mod._hook = hook
        mod.get_axon_ntff_profile_hook = lambda: mod._hook
        mod.set_axon_ntff_profile_hook = lambda h: setattr(mod, "_hook", h)
        sys.modules["antenv.axon_hooks"] = mod
        antenv.axon_hooks = mod
        return True
    except Exception:
        return False


def _run(nc, in_maps, core_ids, label):
    from concourse.bass_utils import run_bass_kernel_spmd
    trace = os.environ.get("KERNEL_TRACE", "0") == "1"
    if trace:
        trace = _ensure_profile_hook()
    kw = {}
    if trace:
        import tempfile
        tdir = tempfile.mkdtemp(prefix=f"ktrace_{label}_")
        kw = {"tmpdir": tdir}
    res = run_bass_kernel_spmd(nc, in_maps, core_ids, trace=trace, **kw)
    _cache.setdefault("exec_ns", {})[label] = res.exec_time_ns
    _cache.setdefault("results_obj", {})[label] = res
    return res.results


# ---------------- main entry ----------------
def kernel(x, W1, b1, W2, b2, gamma, beta, Wih, Whh, bih, bhh, Wl, bl):
    x, W1, b1, W2, b2 = map(np.asarray, (x, W1, b1, W2, b2))
    gamma, beta = np.asarray(gamma), np.asarray(beta)
    Wih, Whh, bih, bhh = map(np.asarray, (Wih, Whh, bih, bhh))
    Wl, bl = np.asarray(Wl), np.asarray(bl)

    if "conv" not in _cache:
        _cache["conv"] = _build_conv_nc()
    if "rnn" not in _cache:
        _cache["rnn"] = _build_rnn_nc()

    # ---- launch A: conv stack over 640 frames on 8 cores ----
    p1, w1t, w2t, b1c, b2c = _prep_conv_inputs(x, W1, b1, W2, b2)
    in_maps = [
        {"p1": p1[k], "w1": w1t, "w2": w2t, "b1": b1c, "b2": b2c}
        for k in range(NCORES)
    ]
    res = _run(_cache["conv"], in_maps, list(range(NCORES)), "conv")
    # ypart [3, NQ, 2parity, npq]: frame f = 2*(npq*h + i) + par
    npq = NPAIR // NQ
    y = np.empty((NF, 3), np.float32)
    for k, r in enumerate(res):
        yp = r["ypart"]
        fr = np.empty((FPC, 3), np.float32)
        for hh in range(NQ):
            for par in range(2):
                idx = 2 * (npq * hh + np.arange(npq)) + par
                fr[idx] = yp[:, hh, par, :].T
        y[k * FPC:(k + 1) * FPC] = fr
    y = y.reshape(B, F, 3)

    # ---- host glue: BN (train-mode) + per-sample channel reorder ----
    mean = y.mean(axis=(0, 2), keepdims=True)
    var = y.var(axis=(0, 2), keepdims=True)
    yn = (y - mean) / np.sqrt(var + EPS) * gamma[None, :, None] \
        + beta[None, :, None]
    t = yn.transpose(0, 2, 1)                        # [B, 3, F]
    rng = t.max(-1) - t.min(-1)
    perm = np.argsort(rng, axis=1, kind="stable")
    tsel = np.take_along_axis(t, perm[:, :, None], axis=1)  # [B, 3, F]

    # ---- launch B: 3 RNNs on 3 cores (+ scaled final linear) ----
    in_maps_b = []
    for r in range(3):
        ts_r = tsel[:, r, :].T                       # [F, B]
        xbv, wht, cfw, wl3 = _prep_rnn_inputs(
            ts_r, Wih[r], Whh[r], bih[r], bhh[r], Wl)
        in_maps_b.append({"xb": xbv, "whht": wht, "cfw": cfw, "wl3": wl3})
    res_b = _run(_cache["rnn"], in_maps_b, [0, 1, 2], "rnn")

    out = res_b[0]["pr"] + res_b[1]["pr"] + res_b[2]["pr"] + bl[None, :]
    return out.astype(np.float32)


# revision 22
# speedup vs baseline: 1.0875x; 1.0161x over previous
"""Trainium2 Bass kernel for nn_NeuralNetwork_31447750541324.

Network: per-frame conv stack (stride==kernel convs -> pure matmuls) ->
BatchNorm1d over (B, len) -> per-sample channel reorder by range ->
3 Elman RNNs (input 1, hidden 256) over F=64 steps -> mean -> linear.

Sharding: launch A runs the conv stack data-parallel over the 640 frames
(80 frames/core on 8 cores).  The tiny [640,3] conv result is re-arranged
on host (BN stats + affine, range argsort, channel select: ~10k FLOPs),
then launch B runs the 3 RNNs on 3 cores (one RNN each) including the
final linear projection; host sums the 3 partial projections + bias.
"""

import os
import numpy as np

# ---------------- static problem dims ----------------
B, F, C, H, W = 10, 64, 3, 180, 180
NF = B * F                      # 640 frames
NCORES = 8
FPC = NF // NCORES              # 80 frames per core
CH, OUT, NCLS = 64, 256, 5
K1, K2 = 9, 9                   # conv1 kernel (9x9, stride 9)
KC1 = C * 9 * 9                 # 243 contraction
KC1P = 256                      # padded to 2 chunks of 128
N1 = 400                        # 20x20 conv1 output positions
EPS = 1e-5

_cache = {}


def _f16(a):
    return np.ascontiguousarray(a, dtype=np.float16)


def _f32(a):
    return np.ascontiguousarray(a, dtype=np.float32)


# ---------------- launch A: conv stack, 8 cores ----------------
# 8-frame DMA groups; 2 frames packed per PSUM tile via column-tiled
# matmuls (partitions 0-63 = even frame, 64-127 = odd frame); ACT does
# relu+bias from PSUM, DVE maxpools in fp16; conv2 split in two halves.
GRP = 8           # frames per DMA group
NGRP = FPC // GRP
NPAIR = FPC // 2  # 40 psum pairs
NQ = 2            # conv2 split into halves


def _build_conv_nc():
    import concourse.bacc as bacc
    import concourse.bass as bass
    import concourse.mybir as mybir
    import concourse.tile as tile

    f16, f32 = mybir.dt.float16, mybir.dt.float32
    nc = bacc.Bacc("TRN2", target_bir_lowering=False, debug=False,
                   num_devices=NCORES)

    p1 = nc.dram_tensor("p1", [NGRP, 128, GRP, 2, N1], f16,
                        kind="ExternalInput")
    w1 = nc.dram_tensor("w1", [128, 2, 128], f16, kind="ExternalInput")
    w2 = nc.dram_tensor("w2", [128, 25, 3], f16, kind="ExternalInput")
    b1 = nc.dram_tensor("b1", [128, 1], f32, kind="ExternalInput")
    b2 = nc.dram_tensor("b2", [3, 1], f32, kind="ExternalInput")
    yp = nc.dram_tensor("ypart", [3, NQ, 2, NPAIR // NQ], f32,
                        kind="ExternalOutput")

    Relu = mybir.ActivationFunctionType.Relu
    X, XY = mybir.AxisListType.X, mybir.AxisListType.XY
    mx = mybir.AluOpType.max

    with tile.TileContext(nc) as tc:
        with (
            tc.tile_pool(name="const", bufs=1) as cp,
            tc.tile_pool(name="frames", bufs=4) as fp,
            tc.tile_pool(name="red", bufs=4) as rp,
            tc.tile_pool(name="ps1", bufs=4, space=bass.MemorySpace.PSUM) as pp1,
            tc.tile_pool(name="ps2", bufs=2, space=bass.MemorySpace.PSUM) as pp2,
        ):
            w1s = cp.tile([128, 2, 128], f16, tag="w1")
            w2s = cp.tile([128, 25, 3], f16, tag="w2")
            b1s = cp.tile([128, 1], f32, tag="b1")
            b2s = cp.tile([3, 1], f32, tag="b2")
            pool1 = cp.tile([128, NPAIR, 100], f16, tag="pool1")
            yo = cp.tile([3, NQ, 2, NPAIR // NQ], f32, tag="yo")
            # consts go on gpsimd's queue so the sync engine's very first
            # instruction is the group-0 frame DMA
            nc.scalar.dma_start(w1s[:], w1[:])
            nc.scalar.dma_start(w2s[:], w2[:])
            nc.scalar.dma_start(b1s[:], b1[:])
            nc.scalar.dma_start(b2s[:], b2[:])

            pv = pool1[:].rearrange("p q (a x b y) -> p q a x b y",
                                    a=2, x=5, b=2)

            def conv2_quarter(h):
                # pairs [10h, 10h+10); even frames from partitions 0:64,
                # odd frames from partitions 64:128 (row-tiled K=64).
                npq = NPAIR // NQ
                sl = slice(npq * h, npq * (h + 1))
                pse = pp2.tile([3, npq, 2, 2], f32, tag="ps2e")
                pso = pp2.tile([3, npq, 2, 2], f32, tag="ps2o")
                for j in range(25):
                    kh, kw = j // 5, j % 5
                    nc.tensor.matmul(pse[:], w2s[0:64, j, :],
                                     pv[0:64, sl, :, kh, :, kw],
                                     start=(j == 0), stop=(j == 24))
                    nc.tensor.matmul(pso[:], w2s[64:128, j, :],
                                     pv[64:128, sl, :, kh, :, kw],
                                     start=(j == 0), stop=(j == 24))
                for par, psx in ((0, pse), (1, pso)):
                    rt2 = rp.tile([3, npq], f32, tag="rt2")
                    nc.vector.tensor_reduce(rt2[:], psx[:], axis=XY, op=mx)
                    nc.scalar.activation(yo[:, h, par, :], rt2[:],
                                         Relu, bias=b2s[:])

            for g in range(NGRP):
                gt = fp.tile([128, GRP, 2, N1], f16, tag="fr")
                nc.sync.dma_start(gt[:, 0:GRP // 2], p1[g, :, 0:GRP // 2])
                nc.sync.dma_start(gt[:, GRP // 2:], p1[g, :, GRP // 2:])
                for p in range(GRP // 2):
                    fa, fb = 2 * p, 2 * p + 1
                    ps = pp1.tile([128, 100, 4], f32, tag="ps")
                    nc.tensor.matmul(ps[0:64], w1s[:, 0, 0:64],
                                     gt[:, fa, 0, :], start=True, stop=False)
                    nc.tensor.matmul(ps[64:128], w1s[:, 0, 64:128],
                                     gt[:, fb, 0, :], start=True, stop=False)
                    nc.tensor.matmul(ps[0:64], w1s[:, 1, 0:64],
                                     gt[:, fa, 1, :], start=False, stop=True)
                    nc.tensor.matmul(ps[64:128], w1s[:, 1, 64:128],
                                     gt[:, fb, 1, :], start=False, stop=True)
                    rt = rp.tile([128, 100], f32, tag="rt")
                    nc.vector.tensor_reduce(rt[:], ps[:], axis=X, op=mx)
                    nc.scalar.activation(pool1[:, g * (GRP // 2) + p, :],
                                         rt[:], Relu, bias=b1s[:])
                if g == NGRP // 2 - 1:
                    conv2_quarter(0)
            conv2_quarter(NQ - 1)
            nc.sync.dma_start(yp[:], yo[:])

    nc.compile()
    return nc


# ---------------- launch B: one RNN per core, 3 cores ----------------
def _build_rnn_nc():
    import concourse.bacc as bacc
    import concourse.bass as bass
    import concourse.mybir as mybir
    import concourse.tile as tile

    f16, f32 = mybir.dt.float16, mybir.dt.float32
    nc = bacc.Bacc("TRN2", target_bir_lowering=False, debug=False,
                   num_devices=3)

    xb = nc.dram_tensor("xb", [128, F, B], f16, kind="ExternalInput")
    wh = nc.dram_tensor("whht", [128, 2, 2, 128], f16, kind="ExternalInput")
    cf = nc.dram_tensor("cfw", [128, 2, 128], f16, kind="ExternalInput")
    wl = nc.dram_tensor("wl3", [128, 2, 5], f16, kind="ExternalInput")
    pr = nc.dram_tensor("pr", [B, NCLS], f32, kind="ExternalOutput")

    Tanh = mybir.ActivationFunctionType.Tanh

    with tile.TileContext(nc) as tc:
        with (
            tc.tile_pool(name="const", bufs=1) as cp,
            tc.tile_pool(name="h", bufs=2) as hp,
            tc.tile_pool(name="ps", bufs=4, space=bass.MemorySpace.PSUM) as pp,
        ):
            xbs = cp.tile([128, F, B], f16, tag="xb")
            whs = cp.tile([128, 2, 2, 128], f16, tag="wh")
            cfs = cp.tile([128, 2, 128], f16, tag="cf")
            wls = cp.tile([128, 2, 5], f16, tag="wl")
            nc.sync.dma_start(xbs[:], xb[:])
            nc.sync.dma_start(whs[:], wh[:])
            nc.sync.dma_start(cfs[:], cf[:])
            nc.sync.dma_start(wls[:], wl[:])

            h = None
            for t in range(F):
                ps = pp.tile([128, 2, B], f32, tag="ps")
                for mc in range(2):
                    nc.tensor.matmul(ps[:, mc, :], cfs[:, mc, :],
                                     xbs[:, t, :], start=True, stop=(t == 0))
                    if t > 0:
                        nc.tensor.matmul(ps[:, mc, :], whs[:, 0, mc, :],
                                         h[:, 0, :], start=False, stop=False)
                        nc.tensor.matmul(ps[:, mc, :], whs[:, 1, mc, :],
                                         h[:, 1, :], start=False, stop=True)
                ht = hp.tile([128, 2, B], f16, tag="h")
                nc.scalar.activation(ht[:], ps[:], Tanh)
                h = ht

            psf = pp.tile([B, NCLS], f32, tag="psf")
            nc.tensor.matmul(psf[:], h[:, 0, :], wls[:, 0, :],
                             start=True, stop=False)
            nc.tensor.matmul(psf[:], h[:, 1, :], wls[:, 1, :],
                             start=False, stop=True)
            po = cp.tile([B, NCLS], f32, tag="po")
            nc.vector.tensor_copy(po[:], psf[:])
            nc.sync.dma_start(pr[:], po[:])

    nc.compile()
    return nc


# ---------------- host-side input prep ----------------
def _prep_conv_inputs(x, W1, b1, W2, b2):
    # im2col for conv1: stride==kernel => non-overlapping patches.
    # n-order (oh10, ow10, ph, pw) groups each 2x2 maxpool window in the
    # last free axis; k-order (c, kh, kw) matches W1 flattening.
    xv = x.reshape(NF, C, 10, 2, 9, 20, 9)          # (fr,c,oh10,ph,kh,w,kw)
    xv = xv.reshape(NF, C, 10, 2, 9, 10, 2, 9)      # split w -> (ow10,pw)
    pat = xv.transpose(0, 1, 4, 7, 2, 5, 3, 6).reshape(NF, KC1, N1)
    patp = np.zeros((NF, KC1P, N1), np.float16)
    patp[:, :KC1] = pat
    # [NF, 128, 2, N1], then group GRP frames per DMA: [NC, NGRP, 128, GRP, 2, N1]
    p1 = patp.reshape(NF, 2, 128, N1).transpose(0, 2, 1, 3)
    p1 = p1.reshape(NCORES, NGRP, GRP, 128, 2, N1).transpose(0, 1, 3, 2, 4, 5)
    p1 = np.ascontiguousarray(p1)

    w1m = np.zeros((KC1P, 64), np.float16)
    w1m[:KC1] = W1.reshape(64, KC1).T               # [K, M]
    w1c = w1m.reshape(2, 128, 64).transpose(1, 0, 2)  # [128, 2, 64]
    w1t = np.concatenate([w1c, w1c], axis=2)        # [128, 2, 128] dup cols
    w1t = np.ascontiguousarray(w1t)

    # conv2 lhsT per (kh,kw): [64, 3], duplicated on rows for odd frames
    w2c = W2.transpose(1, 2, 3, 0).reshape(64, 25, 3).astype(np.float16)
    w2t = np.ascontiguousarray(np.concatenate([w2c, w2c], axis=0))

    b1d = np.concatenate([b1, b1]).reshape(128, 1)
    return p1, w1t, w2t, _f32(b1d), _f32(b2.reshape(3, 1))


def _prep_rnn_inputs(ts_r, Wih_r, Whh_r, bih_r, bhh_r, Wl):
    # ts_r: [F, B] f32 rank-r input sequence
    xbv = np.zeros((128, F, B), np.float16)
    xbv[0] = ts_r
    xbv[1] = 1.0
    wht = np.zeros((128, 2, 2, 128), np.float16)
    WhhT = Whh_r.T                                   # [k, m]
    for kc in range(2):
        for mc in range(2):
            wht[:, kc, mc, :] = WhhT[kc * 128:(kc + 1) * 128,
                                     mc * 128:(mc + 1) * 128]
    cfw = np.zeros((128, 2, 128), np.float16)
    bsum = bih_r + bhh_r
    for mc in range(2):
        cfw[0, mc, :] = Wih_r[mc * 128:(mc + 1) * 128, 0]
        cfw[1, mc, :] = bsum[mc * 128:(mc + 1) * 128]
    wl3 = np.zeros((128, 2, 5), np.float16)
    WlT3 = (Wl.T / 3.0)                              # [256, 5]
    for kc in range(2):
        wl3[:, kc, :] = WlT3[kc * 128:(kc + 1) * 128]
    return xbv, wht, cfw, wl3


def _ensure_profile_hook():
    """antenv.axon_hooks is absent in this image; synthesize it so
    run_bass_kernel_spmd(trace=True) can capture NTFF profiles."""
    import sys
    import types
    try:
        from antenv.axon_hooks import get_axon_ntff_profile_hook  # noqa
        return True
    except ImportError:
        pass
    try:
        sys.path.insert(0, "/root/.axon_site/trn_agent_boot")
        from trn_boot import _ntff_profile_via_ctypes
        hook = _ntff_profile_via_ctypes("/opt/axon/libaxon_pjrt.so")
        if hook is None:
            return False
        import antenv
        mod = types.ModuleType("antenv.axon_hooks")
        mod._hook = hook
        mod.get_axon_ntff_profile_hook = lambda: mod._hook
        mod.set_axon_ntff_profile_hook = lambda h: setattr(mod, "_hook", h)
        sys.modules["antenv.axon_hooks"] = mod
        antenv.axon_hooks = mod
        return True
    except Exception:
        return False


def _run(nc, in_maps, core_ids, label):
    from concourse.bass_utils import run_bass_kernel_spmd
    trace = os.environ.get("KERNEL_TRACE", "0") == "1"
    if trace:
        trace = _ensure_profile_hook()
    kw = {}
    if trace:
        import tempfile
        tdir = tempfile.mkdtemp(prefix=f"ktrace_{label}_")
        kw = {"tmpdir": tdir}
    res = run_bass_kernel_spmd(nc, in_maps, core_ids, trace=trace, **kw)
    _cache.setdefault("exec_ns", {})[label] = res.exec_time_ns
    _cache.setdefault("results_obj", {})[label] = res
    return res.results


# ---------------- main entry ----------------
def kernel(x, W1, b1, W2, b2, gamma, beta, Wih, Whh, bih, bhh, Wl, bl):
    x, W1, b1, W2, b2 = map(np.asarray, (x, W1, b1, W2, b2))
    gamma, beta = np.asarray(gamma), np.asarray(beta)
    Wih, Whh, bih, bhh = map(np.asarray, (Wih, Whh, bih, bhh))
    Wl, bl = np.asarray(Wl), np.asarray(bl)

    if "conv" not in _cache:
        _cache["conv"] = _build_conv_nc()
    if "rnn" not in _cache:
        _cache["rnn"] = _build_rnn_nc()

    # ---- launch A: conv stack over 640 frames on 8 cores ----
    p1, w1t, w2t, b1c, b2c = _prep_conv_inputs(x, W1, b1, W2, b2)
    in_maps = [
        {"p1": p1[k], "w1": w1t, "w2": w2t, "b1": b1c, "b2": b2c}
        for k in range(NCORES)
    ]
    res = _run(_cache["conv"], in_maps, list(range(NCORES)), "conv")
    # ypart [3, NQ, 2parity, npq]: frame f = 2*(npq*h + i) + par
    npq = NPAIR // NQ
    y = np.empty((NF, 3), np.float32)
    for k, r in enumerate(res):
        yp = r["ypart"]
        fr = np.empty((FPC, 3), np.float32)
        for hh in range(NQ):
            for par in range(2):
                idx = 2 * (npq * hh + np.arange(npq)) + par
                fr[idx] = yp[:, hh, par, :].T
        y[k * FPC:(k + 1) * FPC] = fr
    y = y.reshape(B, F, 3)

    # ---- host glue: BN (train-mode) + per-sample channel reorder ----
    mean = y.mean(axis=(0, 2), keepdims=True)
    var = y.var(axis=(0, 2), keepdims=True)
    yn = (y - mean) / np.sqrt(var + EPS) * gamma[None, :, None] \
        + beta[None, :, None]
    t = yn.transpose(0, 2, 1)                        # [B, 3, F]
    rng = t.max(-1) - t.min(-1)
    perm = np.argsort(rng, axis=1, kind="stable")
    tsel = np.take_along_axis(t, perm[:, :, None], axis=1)  # [B, 3, F]

    # ---- launch B: 3 RNNs on 3 cores (+ scaled final linear) ----
    in_maps_b = []
    for r in range(3):
        ts_r = tsel[:, r, :].T                       # [F, B]
        xbv, wht, cfw, wl3 = _prep_rnn_inputs(
            ts_r, Wih[r], Whh[r], bih[r], bhh[r], Wl)
        in_maps_b.append({"xb": xbv, "whht": wht, "cfw": cfw, "wl3": wl3})
    res_b = _run(_cache["rnn"], in_maps_b, [0, 1, 2], "rnn")

    out = res_b[0]["pr"] + res_b[1]["pr"] + res_b[2]["pr"] + bl[None, :]
    return out.astype(np.float32)
